# revision 35
# baseline (speedup 1.0000x reference)
"""MicroHeadAttention Trainium2 kernel (8-core SPMD, data-parallel over
(batch, row-chunk) pairs).

Shapes (hardcoded): x (2, 2048, 1024), weights (1024, 1024), biases (1024,).
EMBED=1024, 16 heads in 2 blocks (g) of 8 micro-heads, head_dim 64.

Decomposition: the reference's "scramble" is a raw row-major reshape, so the
attention head (b, g, m') consumes exactly rows x[b, 256m':256(m'+1)] and
weight columns [512g:512(g+1)], reshaped (256, 512) -> (2048, 64) with
scrambled position n' = 8*row + m (m = 64-channel sub-block).  16 (b, m')
row-chunks across 8 cores = 2 per core; each chunk has g=0,1 -> 4 heads/core.

Per-core dataflow (v2 — engine-balanced schedule):
  warmup:  ~24 dummy matmuls on the mask tile keep the PE busy through the
           DMA preamble so the HAM clock-gate reaches 2.4 GHz before the
           first projection (otherwise the whole kernel runs at K=4/8).
  phase 1: V = x@Wv.T+bv (natural row-major), scrambled to (n', d) layout via
           a DRAM round-trip DMA (with a ones-column appended for the softmax
           denominator).  Q^T/K^T computed channels-on-partitions; the
           mandatory PSUM->SBUF bias copies write strided so qsc/ksc come out
           directly in scrambled (d, n') layout; the two micro-head copies of
           each tile are split between ScalarE and VectorE.
  phase 2: per (p, j5) q-block: S^T = k^T.T @ q^T; causal masks are added by
           the PE itself (identity-weight matmul accumulating the mask tile
           into the S PSUM group) instead of VectorE; exp on ScalarE over
           (128, 1024) double-tiles; ctx^T accumulated as [v | ones].T @ P^T.
           S tiles are emitted one t2-stage ahead of the ctx matmuls so the
           PE never stalls on the exp.  The softmax divide evacuates ctx
           and the denominator rows from PSUM immediately (freeing the ctx
           banks), batches four denominator rows onto partitions 0/32/64/96
           for a single partition-parallel reciprocal per row-chunk, expands
           the rec rows with a tiny PE selector-matmul into PSUM (no gpsimd
           broadcast), and multiplies in place; divide and out-proj emission
           are deferred by a few pipeline steps so their VectorE-dependent
           matmuls never head-of-line-block the next group's S tiles in the
           PE FIFO.
  phase 3: out = ctx^T.T @ Wo^T + bo, interleaved into phase 2: each (p, rc)
           row-block's projection is emitted as soon as its two j5 groups
           finish, filling PE gaps in the ACT-bound attention phase.
"""

import ml_dtypes
import numpy as np

import concourse.bass as bass
import concourse.mybir as mybir
from concourse import bacc
from concourse.tile import TileContext
from concourse.bass_utils import run_bass_kernel_spmd

F32 = mybir.dt.float32
BF16 = mybir.dt.bfloat16
# bf16 matmul operands: same 1 cycle/row as fp32r but LDWEIGHTS gets FWL
# (disabled for 4-byte dtypes), halving the PE's weight-load overhead; all
# accumulation stays fp32 in PSUM.
DT_MM = BF16
NEG = -1e30
E = 1024
R = 512       # rows per core
RP = 256      # rows per pair
ALU = mybir.AluOpType
ACTF = mybir.ActivationFunctionType

_cache = {}


def _build():
    nc = bacc.Bacc()
    xT_d = nc.dram_tensor("xT", (E, R), DT_MM, kind="ExternalInput")
    wq_d = nc.dram_tensor("wqT", (E, E), DT_MM, kind="ExternalInput")
    wk_d = nc.dram_tensor("wkT", (E, E), DT_MM, kind="ExternalInput")
    wv_d = nc.dram_tensor("wvT", (E, E), DT_MM, kind="ExternalInput")
    wo_d = nc.dram_tensor("woTre", (128, 8, E), DT_MM, kind="ExternalInput")
    bq_d = nc.dram_tensor("bqT", (128, 8), F32, kind="ExternalInput")
    bk_d = nc.dram_tensor("bkT8", (128, 8), F32, kind="ExternalInput")
    bv_d = nc.dram_tensor("bvrow", (1, E), F32, kind="ExternalInput")
    bo_d = nc.dram_tensor("borow", (1, E), F32, kind="ExternalInput")
    masks_d = nc.dram_tensor("masks", (128, 4, 512), DT_MM, kind="ExternalInput")
    ident_d = nc.dram_tensor("ident", (128, 128), DT_MM, kind="ExternalInput")
    gsel_d = nc.dram_tensor("gsel", (33, 128), DT_MM, kind="ExternalInput")
    out_d = nc.dram_tensor("out", (R, E), F32, kind="ExternalOutput")

    with TileContext(nc) as tc:
        with (
            tc.tile_pool(name="persist", bufs=1) as pp,
            tc.tile_pool(name="pt", bufs=4) as ptp,
            tc.tile_pool(name="misc", bufs=2) as mp,
            tc.tile_pool(name="dram", bufs=1, space="DRAM") as dp,
        ):
            # ---- persistent tiles ----
            bqT = pp.tile([128, 8], F32, tag="bqT", name="bqT")
            bkT8 = pp.tile([128, 8], F32, tag="bkT8", name="bkT8")
            # m-major in-block coords: position(i | kbl) = 8*(i%64) + 2*kbl
            # + i//64 ; position(c) = 8*(c%64) + c//64 ; masks[i, kbl, c] = 0
            # where pos_k <= pos_q else NEG
            masks = pp.tile([128, 4, 512], DT_MM, tag="masks", name="masks")
            ident = pp.tile([128, 128], DT_MM, tag="ident", name="ident")
            gsel = pp.tile([33, 128], DT_MM, tag="gsel", name="gsel")
            qsc = pp.tile([128, 4096], DT_MM, tag="qsc", name="qsc")
            ksc = pp.tile([128, 4096], DT_MM, tag="ksc", name="ksc")
            vsc = [[pp.tile([128, 16, 65], DT_MM, tag=f"vsc{p}{g}", name=f"vsc{p}{g}")
                    for g in range(2)] for p in range(2)]
            # ctxP[p][c, rc, m, rr] : out-proj lhsT slices are contiguous
            # (FWL needs a single-stride stationary AP); with m-major ctx
            # columns the divide writes 64-contiguous runs into it
            ctxP = [pp.tile([128, 2, 8, 128], DT_MM, tag=f"ctxP{p}", name=f"ctxP{p}")
                    for p in range(2)]
            # persistent rec-row staging (rows 1-31 stay zero so the 33-wide
            # gsel broadcast matmul never reads uninitialized SBUF)
            reck2s = [pp.tile([33, 512], DT_MM, tag=f"reck2{i}",
                              name=f"reck2{i}") for i in range(2)]
            vtmp = dp.tile([2, 2, 2048, 64], DT_MM, tag="vtmp", name="vtmp")

            # masks/ident are host constants so the PE pre-warm only
            # waits on these two small DMAs
            nc.sync.dma_start(masks[:], masks_d[:])
            nc.sync.dma_start(ident[:], ident_d[:])
            nc.sync.dma_start(gsel[:], gsel_d[:])
            nc.sync.dma_start(bqT[:], bq_d[:])
            nc.sync.dma_start(bkT8[:], bk_d[:])
            ones16 = pp.tile([128, 16], F32, tag="ones16", name="ones16")
            nc.gpsimd.memset(ones16[:], 1.0)
            for i in range(2):
                nc.vector.memset(reck2s[i][:], 0.0)
            for p in range(2):
                for g in range(2):
                    nc.vector.tensor_copy(vsc[p][g][:, :, 64], ones16[:])

            with tc.tile_pool(name="stage1", bufs=1) as s1p, \
                 tc.tile_pool(name="ps1", bufs=5, space="PSUM") as psp, \
                 tc.tile_pool(name="pswarm", bufs=1, space="PSUM") as pwp:
                # ---- PE pre-warm: ~24 back-to-back matmuls on the mask
                # tile keep the HAM activity window busy while the input
                # DMAs stream, so the real matmuls start at 2.4 GHz.
                psw = pwp.tile([128, 512], F32, tag="psw", name="psw")
                for _ in range(24):
                    nc.tensor.matmul(psw[:], ident[:], masks[:, 0, :],
                                     start=True, stop=True)

                xt = s1p.tile([128, 8, R], DT_MM, tag="xt", name="xt")
                wq = s1p.tile([128, 8, E], DT_MM, tag="wq", name="wq")
                xT_v = xT_d.rearrange("(ko ki) r -> ki ko r", ki=128)
                wq_v = wq_d.rearrange("(ko ki) o -> ki ko o", ki=128)
                for ko in range(8):
                    nc.sync.dma_start(xt[:, ko], xT_v[:, ko])

                def qk_proj(w_tile, bias_tile, scale, dst):
                    for t in range(8):
                        ps = psp.tile([128, 512], F32, tag="psA", name="psA")
                        for ki in range(8):
                            nc.tensor.matmul(
                                ps[:], w_tile[:, ki, 128 * t:128 * (t + 1)],
                                xt[:, ki, :], start=(ki == 0), stop=(ki == 7))
                        g, u = t // 4, t % 4
                        for mh in range(2):
                            mmv = 2 * u + mh
                            # m-major block layout: contiguous 64-wide runs
                            dest = dst.rearrange(
                                "c (pp j5 m jj) -> c pp j5 m jj",
                                pp=2, j5=4, m=8)[
                                64 * g:64 * (g + 1), :, :, mmv, :]
                            src = ps[64 * mh:64 * (mh + 1), :]
                            # VectorE lanes are partition-locked: it can only
                            # take the copies whose src/dst partition ranges
                            # line up (g == mh); ScalarE handles the crossed
                            # ones.
                            if mh != g:
                                nc.scalar.activation(
                                    dest, src, ACTF.Identity,
                                    bias=bias_tile[64 * mh:64 * (mh + 1), t:t + 1],
                                    scale=scale)
                            else:
                                nc.vector.tensor_scalar(
                                    out=dest, in0=src, scalar1=scale,
                                    scalar2=bias_tile[64 * mh:64 * (mh + 1), t:t + 1],
                                    op0=ALU.mult, op1=ALU.add)

                with tc.tile_pool(name="stagev", bufs=1) as svp:
                    wv = svp.tile([128, 8, E], DT_MM, tag="wv", name="wv")
                    vnat = [svp.tile([128, 2, E], DT_MM, tag=f"vnat{p}", name=f"vnat{p}")
                            for p in range(2)]
                    bvr = svp.tile([1, E], F32, tag="bvr", name="bvr")
                    bv_bc = svp.tile([128, E], F32, tag="bvbc", name="bvbc")
                    nc.sync.dma_start(bvr[:], bv_d[:])
                    nc.gpsimd.partition_broadcast(bv_bc[:], bvr[:])
                    wv_v = wv_d.rearrange("(ko ki) o -> ki ko o", ki=128)
                    # oc-major so V's first output half can start after 2 MB
                    for oc in range(2):
                        for ko in range(8):
                            nc.sync.dma_start(
                                wv[:, ko, 512 * oc:512 * (oc + 1)],
                                wv_v[:, ko, 512 * oc:512 * (oc + 1)])
                    # wq streams behind wv; Q proj runs after the V scramble
                    for ko in range(8):
                        nc.sync.dma_start(wq[:, ko], wq_v[:, ko])

                    for oc in range(2):
                        for rc in range(4):
                            p, half = rc // 2, rc % 2
                            ps = psp.tile([128, 512], F32, tag="psA", name="psA")
                            for ki in range(8):
                                nc.tensor.matmul(
                                    ps[:], xt[:, ki, 128 * rc:128 * (rc + 1)],
                                    wv[:, ki, 512 * oc:512 * (oc + 1)],
                                    start=(ki == 0), stop=(ki == 7))
                            nc.vector.tensor_tensor(
                                vnat[p][:, half, 512 * oc:512 * (oc + 1)],
                                ps[:], bv_bc[:, 512 * oc:512 * (oc + 1)], ALU.add)
                    for p in range(2):
                        for g in range(2):
                            # kb = 8h + 4rb + m//2 ; pin = 64*(m%2) + rr
                            # (row j = 128h + 64rb + rr); the (kb, pin) flat
                            # index is 1024h + 512rb + 64m + rr.  Split per
                            # (h, rb): the DMA balancer tops out at 3 dims.
                            dstv = vtmp[p, g].rearrange(
                                "(h rb m rr) d -> h rb rr m d", h=2, rb=2, m=8)
                            for h in range(2):
                                for rb in range(2):
                                    srcs = vnat[p][64 * rb:64 * (rb + 1), h,
                                                   512 * g:512 * (g + 1)]
                                    nc.sync.dma_start(
                                        dstv[h, rb],
                                        srcs.rearrange("rr (m d) -> rr m d",
                                                       m=8))

                    # Q projection overlaps with the V scramble DMAs
                    qk_proj(wq, bqT, 1.0, qsc)

                # wv/vnat freed; wk reuses that space
                with tc.tile_pool(name="stagek", bufs=1) as skp:
                    wk = skp.tile([128, 8, E], DT_MM, tag="wk", name="wk")
                    wk_v = wk_d.rearrange("(ko ki) o -> ki ko o", ki=128)
                    for ko in range(8):
                        nc.sync.dma_start(wk[:, ko], wk_v[:, ko])
                    for p in range(2):
                        for g in range(2):
                            nc.sync.dma_start(
                                vsc[p][g][:, :, 0:64],
                                vtmp[p, g].rearrange("(kb pin) d -> pin kb d", pin=128))
                    qk_proj(wk, bkT8, 0.125, ksc)

            # stage1 (xt, wq) freed; wo loads into that space
            with tc.tile_pool(name="stageo", bufs=1) as sop:
                wo = sop.tile([128, 8, E], DT_MM, tag="wo", name="wo")
                bor = sop.tile([1, E], F32, tag="bor", name="bor")
                bo_bc = sop.tile([128, E], F32, tag="bobc", name="bobc")
                nc.sync.dma_start(bor[:], bo_d[:])
                nc.gpsimd.partition_broadcast(bo_bc[:], bor[:])
                for ko in range(8):
                    nc.sync.dma_start(wo[:, ko], wo_d[:, ko])

                # ---- attention + interleaved output projection ----
                with tc.tile_pool(name="psS", bufs=2, space="PSUM") as pssp, \
                     tc.tile_pool(name="psctx", bufs=2, space="PSUM") as pcp, \
                     tc.tile_pool(name="psO", bufs=1, space="PSUM") as psop:

                    def out_proj(p, rc):
                        for oc in range(2):
                            ps = psop.tile([128, 512], F32, tag="psO", name="psO")
                            for mmv in range(8):
                                nc.tensor.matmul(
                                    ps[:],
                                    ctxP[p][:, rc, mmv, :],
                                    wo[:, mmv, 512 * oc:512 * (oc + 1)],
                                    start=(mmv == 0), stop=(mmv == 7))
                            outsb = sop.tile([128, 512], F32,
                                             tag=f"outsb{p}{rc}{oc}", name="outsb")
                            nc.vector.tensor_tensor(
                                outsb[:], ps[:],
                                bo_bc[:, 512 * oc:512 * (oc + 1)], ALU.add)
                            nc.sync.dma_start(
                                out_d[RP * p + 128 * rc:RP * p + 128 * (rc + 1),
                                      512 * oc:512 * (oc + 1)],
                                outsb[:])

                    # deferred emissions: [countdown_in_t2_steps, fn];
                    # keeps divide/out-proj matmuls (which wait on VectorE)
                    # from head-of-line-blocking the next group's S tiles
                    pending = []

                    def drain():
                        for item in pending[:]:
                            item[0] -= 1
                            if item[0] <= 0:
                                pending.remove(item)
                                item[1]()

                    pending_tail = []
                    for p in range(2):
                        denS = None
                        for j5 in range(4):
                            jh = j5 % 2
                            if jh == 0:
                                # 4 denominator rows (jh, g) staged on
                                # separate partitions: one partition-parallel
                                # reciprocal per rc instead of four
                                denS = mp.tile([128, 512], F32, tag="denS",
                                               name="denS")
                            nt2 = 2 * (j5 + 1)   # pairs of 128-wide k blocks
                            ctx_ps = [pcp.tile([65, 512], F32, tag="ctxps",
                                               name="ctxps")
                                      for _ in range(2)]
                            sts = [None] * nt2
                            pts = [None] * nt2

                            def s_stage(t2):
                                st = [pssp.tile([128, 1024], F32, tag="st",
                                                name="st") for _ in range(2)]
                                for half in range(2):
                                    kb = 2 * t2 + half
                                    diag = t2 >= 2 * j5
                                    for g in range(2):
                                        nc.tensor.matmul(
                                            st[g][:, 512 * half:512 * (half + 1)],
                                            ksc[64 * g:64 * (g + 1),
                                                2048 * p + 128 * kb:
                                                2048 * p + 128 * (kb + 1)],
                                            qsc[64 * g:64 * (g + 1),
                                                2048 * p + 512 * j5:
                                                2048 * p + 512 * (j5 + 1)],
                                            start=True, stop=not diag)
                                        if diag:
                                            # PE adds the causal mask into the
                                            # accumulation group via identity
                                            nc.tensor.matmul(
                                                st[g][:, 512 * half:512 * (half + 1)],
                                                ident[:],
                                                masks[:, kb - 4 * j5, :],
                                                start=False, stop=True)
                                pt2 = []
                                for g in range(2):
                                    pt = ptp.tile([128, 1024], DT_MM, tag="pt",
                                                  name="pt")
                                    nc.scalar.activation(pt[:], st[g][:], ACTF.Exp)
                                    pt2.append(pt)
                                return st, pt2

                            def ctx_stage(t2, pts=pts, ctx_ps=ctx_ps, p=p,
                                          nt2=nt2):
                                for half in range(2):
                                    kb = 2 * t2 + half
                                    for g in range(2):
                                        nc.tensor.matmul(
                                            ctx_ps[g][:], vsc[p][g][:, kb, :],
                                            pts[t2][g][:, 512 * half:512 * (half + 1)],
                                            start=(kb == 0),
                                            stop=(kb == 2 * nt2 - 1))

                            # one-stage software pipeline: S(t2+1) is on the
                            # PE queue before ctx(t2), so the PE keeps
                            # streaming while the ACT exp catches up
                            for t2 in range(nt2):
                                sts[t2], pts[t2] = s_stage(t2)
                                if t2 == 0 and pending_tail:
                                    # previous group's last ctx + evacuation
                                    # runs under this group's first S tiles
                                    # (fills the pipeline-drain bubble)
                                    pending_tail.pop()()
                                if t2 >= 1:
                                    ctx_stage(t2 - 1)
                                drain()

                            def group_tail(j5=j5, jh=jh, ctx_ps=ctx_ps,
                                           ctx_stage=ctx_stage, nt2=nt2,
                                           denS=denS, p=p):
                                ctx_stage(nt2 - 1)
                                # evacuate PSUM fast (frees the ctx banks for
                                # the next group); the reciprocal/divide runs
                                # later, overlapped under later compute
                                for g in range(2):
                                    # [c, rc, m, 64jh+jj] <- ctx (64m + jj)
                                    nc.vector.tensor_copy(
                                        ctxP[p][64 * g:64 * (g + 1), j5 // 2, :,
                                                64 * jh:64 * (jh + 1)],
                                        ctx_ps[g][0:64, :].rearrange(
                                            "c (m jj) -> c m jj", m=8))
                                    nc.vector.tensor_copy(
                                        denS[32 * (2 * jh + g):
                                             32 * (2 * jh + g) + 1, :],
                                        ctx_ps[g][64:65, :])

                            pending_tail.append(group_tail)

                            if jh == 1:
                                rc = j5 // 2
                                tail_grp = (p == 1 and rc == 1)

                                def divide(p=p, rc=rc, denS=denS,
                                           tail=tail_grp):
                                    recS = mp.tile([128, 512], F32, tag="recS",
                                                   name="recS")
                                    # one partition-parallel reciprocal covers
                                    # all four staged denominator rows
                                    # (0/32/64/96; engine partition bases must
                                    # be 32-aligned)
                                    nc.vector.reciprocal(recS[:], denS[:])
                                    for jh2 in range(2):
                                        # rec rows for both g at partitions
                                        # 0/32, then ONE PE matmul with the
                                        # gsel selector broadcasts them to
                                        # partitions 0-63 / 64-127 in PSUM
                                        reck2 = reck2s[jh2]
                                        for g in range(2):
                                            k4 = 2 * jh2 + g
                                            nc.vector.tensor_copy(
                                                reck2[32 * g:32 * g + 1, :],
                                                recS[32 * k4:32 * k4 + 1, :])
                                        rbc_ps = psop.tile([128, 512], F32,
                                                           tag="rbcps",
                                                           name="rbcps")
                                        if tail and jh2 == 0:
                                            # nothing else queues on the PE
                                            # while the tail reciprocal runs;
                                            # dummy matmuls keep the HAM
                                            # clock-gate warm for the final
                                            # out-projection
                                            for _ in range(10):
                                                nc.tensor.matmul(
                                                    rbc_ps[:], ident[:],
                                                    masks[:, 0, :],
                                                    start=True, stop=True)
                                        nc.tensor.matmul(
                                            rbc_ps[:], gsel[:],
                                            reck2[0:33, :],
                                            start=True, stop=True)
                                        for g in range(2):
                                            dst = ctxP[p][64 * g:64 * (g + 1),
                                                          rc, :,
                                                          64 * jh2:64 * (jh2 + 1)]
                                            nc.vector.tensor_tensor(
                                                dst, dst,
                                                rbc_ps[64 * g:64 * (g + 1), :]
                                                .rearrange(
                                                    "c (m jj) -> c m jj", m=8),
                                                ALU.mult)

                                pending.append([3, divide])
                                pending.append(
                                    [6, lambda p=p, rc=rc: out_proj(p, rc)])
                    while pending_tail:
                        pending_tail.pop()()
                    for item in pending:
                        item[1]()

    nc.compile()
    return nc


def _get_nc():
    key = "nc"
    if key not in _cache:
        _cache[key] = _build()
    return _cache[key]


def pack_in_maps(x, Wq, bq, Wk, bk, Wv, bv, Wo, bo):
    BF = ml_dtypes.bfloat16
    x = np.asarray(x, np.float32)
    WqT = np.ascontiguousarray(np.asarray(Wq, np.float32).T.astype(BF))
    WkT = np.ascontiguousarray(np.asarray(Wk, np.float32).T.astype(BF))
    WvT = np.ascontiguousarray(np.asarray(Wv, np.float32).T.astype(BF))
    # woTre[64g + d, m, o] = Wo[o, 512g + 64m + d]
    WoTre = np.ascontiguousarray(
        np.asarray(Wo, np.float32).T.reshape(2, 8, 64, E).transpose(0, 2, 1, 3)
        .reshape(128, 8, E).astype(BF))
    bqT = np.ascontiguousarray(np.asarray(bq, np.float32).reshape(8, 128).T)
    bkT8 = np.ascontiguousarray((np.asarray(bk, np.float32) / 8.0).reshape(8, 128).T)
    bvrow = np.asarray(bv, np.float32).reshape(1, E)
    borow = np.asarray(bo, np.float32).reshape(1, E)
    # m-major in-block coords; masks[i, kbl, c] = 0 where
    # 8*(i%64) + 2*kbl + i//64 <= 8*(c%64) + c//64 else NEG
    ii = np.arange(128)[:, None, None]
    kbl = np.arange(4)[None, :, None]
    cc = np.arange(512)[None, None, :]
    pos_k = 8 * (ii % 64) + 2 * kbl + ii // 64
    pos_q = 8 * (cc % 64) + cc // 64
    masks = np.where(pos_k <= pos_q, 0.0, NEG).astype(BF)
    ident = np.eye(128).astype(BF)
    gsel = np.zeros((33, 128), np.float32)
    gsel[0, 0:64] = 1.0
    gsel[32, 64:128] = 1.0
    gsel = gsel.astype(BF)

    in_maps = []
    for c in range(8):
        xTs = np.empty((E, R), BF)
        for p in range(2):
            h = 2 * c + p
            b_, mp_ = divmod(h, 8)
            xTs[:, RP * p:RP * (p + 1)] = x[b_, RP * mp_:RP * (mp_ + 1), :].T.astype(BF)
        in_maps.append({
            "xT": np.ascontiguousarray(xTs), "wqT": WqT, "wkT": WkT,
            "wvT": WvT, "woTre": WoTre, "bqT": bqT, "bkT8": bkT8,
            "bvrow": bvrow, "borow": borow, "masks": masks, "ident": ident,
            "gsel": gsel,
        })
    return in_maps


def unpack_out(results):
    out = np.empty((2, 2048, E), np.float32)
    for c in range(8):
        o = results[c]["out"]
        for p in range(2):
            h = 2 * c + p
            b_, mp_ = divmod(h, 8)
            out[b_, RP * mp_:RP * (mp_ + 1), :] = o[RP * p:RP * (p + 1), :]
    return out


def kernel(x, Wq, bq, Wk, bk, Wv, bv, Wo, bo):
    in_maps = pack_in_maps(x, Wq, bq, Wk, bk, Wv, bv, Wo, bo)
    nc = _get_nc()
    res = run_bass_kernel_spmd(nc, in_maps, core_ids=list(range(8)))
    return unpack_out(res.results)


# revision 38
# speedup vs baseline: 1.0172x; 1.0172x over previous
"""MicroHeadAttention Trainium2 kernel (8-core SPMD, data-parallel over
(batch, row-chunk) pairs).

Shapes (hardcoded): x (2, 2048, 1024), weights (1024, 1024), biases (1024,).
EMBED=1024, 16 heads in 2 blocks (g) of 8 micro-heads, head_dim 64.

Decomposition: the reference's "scramble" is a raw row-major reshape, so the
attention head (b, g, m') consumes exactly rows x[b, 256m':256(m'+1)] and
weight columns [512g:512(g+1)], reshaped (256, 512) -> (2048, 64) with
scrambled position n' = 8*row + m (m = 64-channel sub-block).  16 (b, m')
row-chunks across 8 cores = 2 per core; each chunk has g=0,1 -> 4 heads/core.

Per-core dataflow (v2 — engine-balanced schedule):
  warmup:  ~24 dummy matmuls on the mask tile keep the PE busy through the
           DMA preamble so the HAM clock-gate reaches 2.4 GHz before the
           first projection (otherwise the whole kernel runs at K=4/8).
  phase 1: V = x@Wv.T+bv (natural row-major), scrambled to (n', d) layout via
           a DRAM round-trip DMA (with a ones-column appended for the softmax
           denominator).  Q^T/K^T computed channels-on-partitions; the
           mandatory PSUM->SBUF bias copies write strided so qsc/ksc come out
           directly in scrambled (d, n') layout; the two micro-head copies of
           each tile are split between ScalarE and VectorE.
  phase 2: per (p, j5) q-block: S^T = k^T.T @ q^T; causal masks are added by
           the PE itself (identity-weight matmul accumulating the mask tile
           into the S PSUM group) instead of VectorE; exp on ScalarE over
           (128, 1024) double-tiles; ctx^T accumulated as [v | ones].T @ P^T.
           S tiles are emitted one t2-stage ahead of the ctx matmuls so the
           PE never stalls on the exp.  The softmax divide evacuates ctx
           and the denominator rows from PSUM immediately (freeing the ctx
           banks), batches four denominator rows onto partitions 0/32/64/96
           for a single partition-parallel reciprocal per row-chunk, expands
           the rec rows with a tiny PE selector-matmul into PSUM (no gpsimd
           broadcast), and multiplies in place; divide and out-proj emission
           are deferred by a few pipeline steps so their VectorE-dependent
           matmuls never head-of-line-block the next group's S tiles in the
           PE FIFO.
  phase 3: out = ctx^T.T @ Wo^T + bo, interleaved into phase 2: each (p, rc)
           row-block's projection is emitted as soon as its two j5 groups
           finish, filling PE gaps in the ACT-bound attention phase.
"""

import ml_dtypes
import numpy as np

import concourse.bass as bass
import concourse.mybir as mybir
from concourse import bacc
from concourse.tile import TileContext
from concourse.bass_utils import run_bass_kernel_spmd

F32 = mybir.dt.float32
BF16 = mybir.dt.bfloat16
# bf16 matmul operands: same 1 cycle/row as fp32r but LDWEIGHTS gets FWL
# (disabled for 4-byte dtypes), halving the PE's weight-load overhead; all
# accumulation stays fp32 in PSUM.
DT_MM = BF16
NEG = -1e30
E = 1024
R = 512       # rows per core
RP = 256      # rows per pair
ALU = mybir.AluOpType
ACTF = mybir.ActivationFunctionType

_cache = {}


def _build():
    nc = bacc.Bacc()
    xT_d = nc.dram_tensor("xT", (E, R), DT_MM, kind="ExternalInput")
    wq_d = nc.dram_tensor("wqT", (E, E), DT_MM, kind="ExternalInput")
    wk_d = nc.dram_tensor("wkT", (E, E), DT_MM, kind="ExternalInput")
    wv_d = nc.dram_tensor("wvT", (E, E), DT_MM, kind="ExternalInput")
    wo_d = nc.dram_tensor("woTre", (128, 8, E), DT_MM, kind="ExternalInput")
    bq_d = nc.dram_tensor("bqT", (128, 8), F32, kind="ExternalInput")
    bk_d = nc.dram_tensor("bkT8", (128, 8), F32, kind="ExternalInput")
    bv_d = nc.dram_tensor("bvrow", (1, E), F32, kind="ExternalInput")
    bo_d = nc.dram_tensor("borow", (1, E), F32, kind="ExternalInput")
    masks_d = nc.dram_tensor("masks", (128, 4, 512), DT_MM, kind="ExternalInput")
    ident_d = nc.dram_tensor("ident", (128, 128), DT_MM, kind="ExternalInput")
    gsel_d = nc.dram_tensor("gsel", (33, 128), DT_MM, kind="ExternalInput")
    out_d = nc.dram_tensor("out", (R, E), F32, kind="ExternalOutput")

    with TileContext(nc) as tc:
        with (
            tc.tile_pool(name="persist", bufs=1) as pp,
            tc.tile_pool(name="pt", bufs=4) as ptp,
            tc.tile_pool(name="misc", bufs=2) as mp,
            tc.tile_pool(name="dram", bufs=1, space="DRAM") as dp,
        ):
            # ---- persistent tiles ----
            bqT = pp.tile([128, 8], F32, tag="bqT", name="bqT")
            bkT8 = pp.tile([128, 8], F32, tag="bkT8", name="bkT8")
            # m-major in-block coords: position(i | kbl) = 8*(i%64) + 2*kbl
            # + i//64 ; position(c) = 8*(c%64) + c//64 ; masks[i, kbl, c] = 0
            # where pos_k <= pos_q else NEG
            masks = pp.tile([128, 4, 512], DT_MM, tag="masks", name="masks")
            ident = pp.tile([128, 128], DT_MM, tag="ident", name="ident")
            gsel = pp.tile([33, 128], DT_MM, tag="gsel", name="gsel")
            qsc = pp.tile([128, 4096], DT_MM, tag="qsc", name="qsc")
            ksc = pp.tile([128, 4096], DT_MM, tag="ksc", name="ksc")
            vsc = [[pp.tile([128, 16, 65], DT_MM, tag=f"vsc{p}{g}", name=f"vsc{p}{g}")
                    for g in range(2)] for p in range(2)]
            # ctxP[p][c, rc, m, rr] : out-proj lhsT slices are contiguous
            # (FWL needs a single-stride stationary AP); with m-major ctx
            # columns the divide writes 64-contiguous runs into it
            ctxP = [pp.tile([128, 2, 8, 128], DT_MM, tag=f"ctxP{p}", name=f"ctxP{p}")
                    for p in range(2)]
            # persistent rec-row staging (rows 1-31 stay zero so the 33-wide
            # gsel broadcast matmul never reads uninitialized SBUF)
            reck2s = [pp.tile([33, 512], DT_MM, tag=f"reck2{i}",
                              name=f"reck2{i}") for i in range(2)]
            vtmp = dp.tile([2, 2, 2048, 64], DT_MM, tag="vtmp", name="vtmp")

            # masks/ident are host constants so the PE pre-warm only
            # waits on these two small DMAs
            nc.sync.dma_start(masks[:], masks_d[:])
            nc.sync.dma_start(ident[:], ident_d[:])
            nc.sync.dma_start(gsel[:], gsel_d[:])
            nc.sync.dma_start(bqT[:], bq_d[:])
            nc.sync.dma_start(bkT8[:], bk_d[:])
            ones16 = pp.tile([128, 16], F32, tag="ones16", name="ones16")
            nc.gpsimd.memset(ones16[:], 1.0)
            for i in range(2):
                nc.vector.memset(reck2s[i][:], 0.0)
            for p in range(2):
                for g in range(2):
                    nc.vector.tensor_copy(vsc[p][g][:, :, 64], ones16[:])

            with tc.tile_pool(name="stage1", bufs=1) as s1p, \
                 tc.tile_pool(name="ps1", bufs=5, space="PSUM") as psp, \
                 tc.tile_pool(name="pswarm", bufs=1, space="PSUM") as pwp:
                # ---- PE pre-warm: ~24 back-to-back matmuls on the mask
                # tile keep the HAM activity window busy while the input
                # DMAs stream, so the real matmuls start at 2.4 GHz.
                psw = pwp.tile([128, 512], F32, tag="psw", name="psw")
                for _ in range(24):
                    nc.tensor.matmul(psw[:], ident[:], masks[:, 0, :],
                                     start=True, stop=True)

                xt = s1p.tile([128, 8, R], DT_MM, tag="xt", name="xt")
                wq = s1p.tile([128, 8, E], DT_MM, tag="wq", name="wq")
                xT_v = xT_d.rearrange("(ko ki) r -> ki ko r", ki=128)
                wq_v = wq_d.rearrange("(ko ki) o -> ki ko o", ki=128)
                for ko in range(8):
                    nc.sync.dma_start(xt[:, ko], xT_v[:, ko])

                def qk_proj(w_tile, bias_tile, scale, dst):
                    for t in range(8):
                        ps = psp.tile([128, 512], F32, tag="psA", name="psA")
                        for ki in range(8):
                            nc.tensor.matmul(
                                ps[:], w_tile[:, ki, 128 * t:128 * (t + 1)],
                                xt[:, ki, :], start=(ki == 0), stop=(ki == 7))
                        g, u = t // 4, t % 4
                        for mh in range(2):
                            mmv = 2 * u + mh
                            # m-major block layout: contiguous 64-wide runs
                            dest = dst.rearrange(
                                "c (pp j5 m jj) -> c pp j5 m jj",
                                pp=2, j5=4, m=8)[
                                64 * g:64 * (g + 1), :, :, mmv, :]
                            src = ps[64 * mh:64 * (mh + 1), :]
                            # VectorE lanes are partition-locked: it can only
                            # take the copies whose src/dst partition ranges
                            # line up (g == mh); ScalarE handles the crossed
                            # ones.
                            if mh != g:
                                nc.scalar.activation(
                                    dest, src, ACTF.Identity,
                                    bias=bias_tile[64 * mh:64 * (mh + 1), t:t + 1],
                                    scale=scale)
                            else:
                                nc.vector.tensor_scalar(
                                    out=dest, in0=src, scalar1=scale,
                                    scalar2=bias_tile[64 * mh:64 * (mh + 1), t:t + 1],
                                    op0=ALU.mult, op1=ALU.add)

                with tc.tile_pool(name="stagev", bufs=1) as svp:
                    wv = svp.tile([128, 8, E], DT_MM, tag="wv", name="wv")
                    vnat = [svp.tile([128, 2, E], DT_MM, tag=f"vnat{p}", name=f"vnat{p}")
                            for p in range(2)]
                    bvr = svp.tile([1, E], F32, tag="bvr", name="bvr")
                    bv_bc = svp.tile([128, E], F32, tag="bvbc", name="bvbc")
                    nc.sync.dma_start(bvr[:], bv_d[:])
                    nc.gpsimd.partition_broadcast(bv_bc[:], bvr[:])
                    wv_v = wv_d.rearrange("(ko ki) o -> ki ko o", ki=128)
                    # oc-major so V's first output half can start after 2 MB
                    for oc in range(2):
                        for ko in range(8):
                            nc.sync.dma_start(
                                wv[:, ko, 512 * oc:512 * (oc + 1)],
                                wv_v[:, ko, 512 * oc:512 * (oc + 1)])
                    # wq streams behind wv; Q proj runs after the V scramble
                    for ko in range(8):
                        nc.sync.dma_start(wq[:, ko], wq_v[:, ko])

                    for oc in range(2):
                        for rc in range(4):
                            p, half = rc // 2, rc % 2
                            ps = psp.tile([128, 512], F32, tag="psA", name="psA")
                            for ki in range(8):
                                nc.tensor.matmul(
                                    ps[:], xt[:, ki, 128 * rc:128 * (rc + 1)],
                                    wv[:, ki, 512 * oc:512 * (oc + 1)],
                                    start=(ki == 0), stop=(ki == 7))
                            nc.vector.tensor_tensor(
                                vnat[p][:, half, 512 * oc:512 * (oc + 1)],
                                ps[:], bv_bc[:, 512 * oc:512 * (oc + 1)], ALU.add)
                    for p in range(2):
                        for g in range(2):
                            # kb = 8h + 4rb + m//2 ; pin = 64*(m%2) + rr
                            # (row j = 128h + 64rb + rr); the (kb, pin) flat
                            # index is 1024h + 512rb + 64m + rr.  Split per
                            # (h, rb): the DMA balancer tops out at 3 dims.
                            dstv = vtmp[p, g].rearrange(
                                "(h rb m rr) d -> h rb rr m d", h=2, rb=2, m=8)
                            for h in range(2):
                                for rb in range(2):
                                    srcs = vnat[p][64 * rb:64 * (rb + 1), h,
                                                   512 * g:512 * (g + 1)]
                                    nc.sync.dma_start(
                                        dstv[h, rb],
                                        srcs.rearrange("rr (m d) -> rr m d",
                                                       m=8))

                    # Q projection overlaps with the V scramble DMAs
                    qk_proj(wq, bqT, 1.0, qsc)

                # wv/vnat freed; wk reuses that space
                with tc.tile_pool(name="stagek", bufs=1) as skp:
                    wk = skp.tile([128, 8, E], DT_MM, tag="wk", name="wk")
                    wk_v = wk_d.rearrange("(ko ki) o -> ki ko o", ki=128)
                    for ko in range(8):
                        nc.sync.dma_start(wk[:, ko], wk_v[:, ko])
                    for p in range(2):
                        for g in range(2):
                            nc.sync.dma_start(
                                vsc[p][g][:, :, 0:64],
                                vtmp[p, g].rearrange("(kb pin) d -> pin kb d", pin=128))
                    qk_proj(wk, bkT8, 0.125, ksc)

            # stage1 (xt, wq) freed; wo loads into that space
            with tc.tile_pool(name="stageo", bufs=1) as sop:
                wo = sop.tile([128, 8, E], DT_MM, tag="wo", name="wo")
                bor = sop.tile([1, E], F32, tag="bor", name="bor")
                bo_bc = sop.tile([128, E], F32, tag="bobc", name="bobc")
                nc.sync.dma_start(bor[:], bo_d[:])
                nc.gpsimd.partition_broadcast(bo_bc[:], bor[:])
                for ko in range(8):
                    nc.sync.dma_start(wo[:, ko], wo_d[:, ko])

                # ---- attention + interleaved output projection ----
                with tc.tile_pool(name="psS", bufs=2, space="PSUM") as pssp, \
                     tc.tile_pool(name="psctx", bufs=2, space="PSUM") as pcp, \
                     tc.tile_pool(name="psO", bufs=1, space="PSUM") as psop:

                    def out_proj(p, rc):
                        for oc in range(2):
                            ps = psop.tile([128, 512], F32, tag="psO", name="psO")
                            for mmv in range(8):
                                nc.tensor.matmul(
                                    ps[:],
                                    ctxP[p][:, rc, mmv, :],
                                    wo[:, mmv, 512 * oc:512 * (oc + 1)],
                                    start=(mmv == 0), stop=(mmv == 7))
                            outsb = sop.tile([128, 512], F32,
                                             tag=f"outsb{p}{rc}{oc}", name="outsb")
                            nc.vector.tensor_tensor(
                                outsb[:], ps[:],
                                bo_bc[:, 512 * oc:512 * (oc + 1)], ALU.add)
                            nc.sync.dma_start(
                                out_d[RP * p + 128 * rc:RP * p + 128 * (rc + 1),
                                      512 * oc:512 * (oc + 1)],
                                outsb[:])

                    # deferred emissions: [countdown_in_t2_steps, fn];
                    # keeps divide/out-proj matmuls (which wait on VectorE)
                    # from head-of-line-blocking the next group's S tiles
                    pending = []

                    def drain():
                        for item in pending[:]:
                            item[0] -= 1
                            if item[0] <= 0:
                                pending.remove(item)
                                item[1]()

                    pending_tail = []
                    for p in range(2):
                        denS = None
                        # descending j5: pairs (3,2) then (1,0), so each
                        # pair-completion divide is covered by a LONG next
                        # group (the p-transition lands on nt2=8, not 2)
                        for j5 in (3, 2, 1, 0):
                            jh = j5 % 2
                            if jh == 1:
                                # 4 denominator rows (jh, g) staged on
                                # separate partitions: one partition-parallel
                                # reciprocal per rc instead of four
                                denS = mp.tile([128, 512], F32, tag="denS",
                                               name="denS")
                            nt2 = 2 * (j5 + 1)   # pairs of 128-wide k blocks
                            ctx_ps = [pcp.tile([65, 512], F32, tag="ctxps",
                                               name="ctxps")
                                      for _ in range(2)]
                            sts = [None] * nt2
                            pts = [None] * nt2

                            def s_stage(t2):
                                st = [pssp.tile([128, 1024], F32, tag="st",
                                                name="st") for _ in range(2)]
                                for half in range(2):
                                    kb = 2 * t2 + half
                                    diag = t2 >= 2 * j5
                                    for g in range(2):
                                        nc.tensor.matmul(
                                            st[g][:, 512 * half:512 * (half + 1)],
                                            ksc[64 * g:64 * (g + 1),
                                                2048 * p + 128 * kb:
                                                2048 * p + 128 * (kb + 1)],
                                            qsc[64 * g:64 * (g + 1),
                                                2048 * p + 512 * j5:
                                                2048 * p + 512 * (j5 + 1)],
                                            start=True, stop=not diag)
                                        if diag:
                                            # PE adds the causal mask into the
                                            # accumulation group via identity
                                            nc.tensor.matmul(
                                                st[g][:, 512 * half:512 * (half + 1)],
                                                ident[:],
                                                masks[:, kb - 4 * j5, :],
                                                start=False, stop=True)
                                pt2 = []
                                for g in range(2):
                                    pt = ptp.tile([128, 1024], DT_MM, tag="pt",
                                                  name="pt")
                                    nc.scalar.activation(pt[:], st[g][:], ACTF.Exp)
                                    pt2.append(pt)
                                return st, pt2

                            def ctx_stage(t2, pts=pts, ctx_ps=ctx_ps, p=p,
                                          nt2=nt2):
                                for half in range(2):
                                    kb = 2 * t2 + half
                                    for g in range(2):
                                        nc.tensor.matmul(
                                            ctx_ps[g][:], vsc[p][g][:, kb, :],
                                            pts[t2][g][:, 512 * half:512 * (half + 1)],
                                            start=(kb == 0),
                                            stop=(kb == 2 * nt2 - 1))

                            # one-stage software pipeline: S(t2+1) is on the
                            # PE queue before ctx(t2), so the PE keeps
                            # streaming while the ACT exp catches up
                            for t2 in range(nt2):
                                sts[t2], pts[t2] = s_stage(t2)
                                if t2 == 0 and pending_tail:
                                    # previous group's last ctx + evacuation
                                    # runs under this group's first S tiles
                                    # (fills the pipeline-drain bubble)
                                    pending_tail.pop()()
                                if t2 >= 1:
                                    ctx_stage(t2 - 1)
                                drain()

                            def group_tail(j5=j5, jh=jh, ctx_ps=ctx_ps,
                                           ctx_stage=ctx_stage, nt2=nt2,
                                           denS=denS, p=p):
                                ctx_stage(nt2 - 1)
                                # evacuate PSUM fast (frees the ctx banks for
                                # the next group); the reciprocal/divide runs
                                # later, overlapped under later compute
                                for g in range(2):
                                    # [c, rc, m, 64jh+jj] <- ctx (64m + jj)
                                    nc.vector.tensor_copy(
                                        ctxP[p][64 * g:64 * (g + 1), j5 // 2, :,
                                                64 * jh:64 * (jh + 1)],
                                        ctx_ps[g][0:64, :].rearrange(
                                            "c (m jj) -> c m jj", m=8))
                                    nc.vector.tensor_copy(
                                        denS[32 * (2 * jh + g):
                                             32 * (2 * jh + g) + 1, :],
                                        ctx_ps[g][64:65, :])

                            pending_tail.append(group_tail)

                            if jh == 0:
                                rc = j5 // 2
                                tail_grp = (p == 1 and rc == 0)

                                def divide(p=p, rc=rc, denS=denS,
                                           tail=tail_grp):
                                    recS = mp.tile([128, 512], F32, tag="recS",
                                                   name="recS")
                                    # one partition-parallel reciprocal covers
                                    # all four staged denominator rows
                                    # (0/32/64/96; engine partition bases must
                                    # be 32-aligned)
                                    nc.vector.reciprocal(recS[:], denS[:])
                                    for jh2 in range(2):
                                        # rec rows for both g at partitions
                                        # 0/32, then ONE PE matmul with the
                                        # gsel selector broadcasts them to
                                        # partitions 0-63 / 64-127 in PSUM
                                        reck2 = reck2s[jh2]
                                        for g in range(2):
                                            k4 = 2 * jh2 + g
                                            nc.vector.tensor_copy(
                                                reck2[32 * g:32 * g + 1, :],
                                                recS[32 * k4:32 * k4 + 1, :])
                                        rbc_ps = psop.tile([128, 512], F32,
                                                           tag="rbcps",
                                                           name="rbcps")
                                        if tail and jh2 == 0:
                                            # nothing else queues on the PE
                                            # while the tail reciprocal runs;
                                            # dummy matmuls keep the HAM
                                            # clock-gate warm for the final
                                            # out-projection
                                            for _ in range(10):
                                                nc.tensor.matmul(
                                                    rbc_ps[:], ident[:],
                                                    masks[:, 0, :],
                                                    start=True, stop=True)
                                        nc.tensor.matmul(
                                            rbc_ps[:], gsel[:],
                                            reck2[0:33, :],
                                            start=True, stop=True)
                                        for g in range(2):
                                            dst = ctxP[p][64 * g:64 * (g + 1),
                                                          rc, :,
                                                          64 * jh2:64 * (jh2 + 1)]
                                            nc.vector.tensor_tensor(
                                                dst, dst,
                                                rbc_ps[64 * g:64 * (g + 1), :]
                                                .rearrange(
                                                    "c (m jj) -> c m jj", m=8),
                                                ALU.mult)

                                pending.append([3, divide])
                                pending.append(
                                    [6, lambda p=p, rc=rc: out_proj(p, rc)])
                    while pending_tail:
                        pending_tail.pop()()
                    for item in pending:
                        item[1]()

    nc.compile()
    return nc


def _get_nc():
    key = "nc"
    if key not in _cache:
        _cache[key] = _build()
    return _cache[key]


def pack_in_maps(x, Wq, bq, Wk, bk, Wv, bv, Wo, bo):
    BF = ml_dtypes.bfloat16
    x = np.asarray(x, np.float32)
    WqT = np.ascontiguousarray(np.asarray(Wq, np.float32).T.astype(BF))
    WkT = np.ascontiguousarray(np.asarray(Wk, np.float32).T.astype(BF))
    WvT = np.ascontiguousarray(np.asarray(Wv, np.float32).T.astype(BF))
    # woTre[64g + d, m, o] = Wo[o, 512g + 64m + d]
    WoTre = np.ascontiguousarray(
        np.asarray(Wo, np.float32).T.reshape(2, 8, 64, E).transpose(0, 2, 1, 3)
        .reshape(128, 8, E).astype(BF))
    bqT = np.ascontiguousarray(np.asarray(bq, np.float32).reshape(8, 128).T)
    bkT8 = np.ascontiguousarray((np.asarray(bk, np.float32) / 8.0).reshape(8, 128).T)
    bvrow = np.asarray(bv, np.float32).reshape(1, E)
    borow = np.asarray(bo, np.float32).reshape(1, E)
    # m-major in-block coords; masks[i, kbl, c] = 0 where
    # 8*(i%64) + 2*kbl + i//64 <= 8*(c%64) + c//64 else NEG
    ii = np.arange(128)[:, None, None]
    kbl = np.arange(4)[None, :, None]
    cc = np.arange(512)[None, None, :]
    pos_k = 8 * (ii % 64) + 2 * kbl + ii // 64
    pos_q = 8 * (cc % 64) + cc // 64
    masks = np.where(pos_k <= pos_q, 0.0, NEG).astype(BF)
    ident = np.eye(128).astype(BF)
    gsel = np.zeros((33, 128), np.float32)
    gsel[0, 0:64] = 1.0
    gsel[32, 64:128] = 1.0
    gsel = gsel.astype(BF)

    in_maps = []
    for c in range(8):
        xTs = np.empty((E, R), BF)
        for p in range(2):
            h = 2 * c + p
            b_, mp_ = divmod(h, 8)
            xTs[:, RP * p:RP * (p + 1)] = x[b_, RP * mp_:RP * (mp_ + 1), :].T.astype(BF)
        in_maps.append({
            "xT": np.ascontiguousarray(xTs), "wqT": WqT, "wkT": WkT,
            "wvT": WvT, "woTre": WoTre, "bqT": bqT, "bkT8": bkT8,
            "bvrow": bvrow, "borow": borow, "masks": masks, "ident": ident,
            "gsel": gsel,
        })
    return in_maps


def unpack_out(results):
    out = np.empty((2, 2048, E), np.float32)
    for c in range(8):
        o = results[c]["out"]
        for p in range(2):
            h = 2 * c + p
            b_, mp_ = divmod(h, 8)
            out[b_, RP * mp_:RP * (mp_ + 1), :] = o[RP * p:RP * (p + 1), :]
    return out


def kernel(x, Wq, bq, Wk, bk, Wv, bv, Wo, bo):
    in_maps = pack_in_maps(x, Wq, bq, Wk, bk, Wv, bv, Wo, bo)
    nc = _get_nc()
    res = run_bass_kernel_spmd(nc, in_maps, core_ids=list(range(8)))
    return unpack_out(res.results)
